# revision 16
# baseline (speedup 1.0000x reference)
"""Trainium2 kernel for DigitConvolutionalModel.

Model: x(B,784) -> reshape(28,28) -> conv3x3 'VALID' (cross-correlation)
       -> flatten(676) -> Linear(676,256)+ReLU -> Linear(256,10).

The conv is linear, so it folds into the first Linear:
    feat = x @ Wc          (Wc: 784x676 sparse conv matrix)
    h    = relu(feat @ w1 + b1) = relu(x @ (Wc @ w1) + b1)
Device work is then two GEMMs per batch tile:
    H^T = relu(W_eff^T-tiles . x^T + b1);  out^T = w2^T . H + b2

Sharding: pure data parallel over 8 cores (8192 rows each). The host
pre-transposes each shard to x^T (contraction dim on SBUF partitions) and
casts to bf16 so the PE streams it directly; weights are replicated.

Perf structure (vs the 82us v1 baseline):
  - K tiled 6x128 + 16-row tail (was 7x112).  The two 16-row tail
    matmuls (one per 128-out m-tile) sit on disjoint PE row groups
    (partitions 0-15 / 32-47) so they execute concurrently.
  - GEMM2's two [128k,16m] matmuls go to disjoint col groups (PSUM
    partitions 0-15 / 32-47) and run concurrently; halves are summed
    on DVE, biased on ACT.
  - PE warm-up matmuls bridge the startup DMA wait so the HAM clock
    gate reaches 2.4 GHz before real work arrives.
  - One 3D DMA per chunk loads all six x k-tiles (x^T is shipped as
    [128, 6, b_shard]); few DMA queues keeps the framework's
    per-queue semaphore setup/teardown short.
  - First chunks are 256/512 columns so the first matmul starts as
    early as possible; GEMM2 pipelines one 512-block behind GEMM1.
"""

import os
from contextlib import ExitStack

import numpy as np
import ml_dtypes

import concourse.bass as bass
import concourse.tile as tile
from concourse import bacc, mybir
from concourse.bass_utils import run_bass_kernel_spmd

N_CORES = 8
B = 65536
B_SHARD = B // N_CORES  # 8192
K = 784                 # contraction dim (pixels)
KT = 128                # main k-tile partition size
NKT = 6                 # 6*128 = 768 main rows
KTAIL = K - NKT * KT    # 16 tail rows
CH = 256                # hidden channels
MT = 128                # m-tile (output channels per matmul)
NMT = CH // MT
OUT_CH = 10
OUT_PAD = 16            # padded output channels
SUB = 512               # max matmul moving free dim / PSUM bank
OGRP = 2048             # output store granularity (batch columns)
N_WARM = 15             # PE warm-up matmuls (N=256 each)
BF16 = mybir.dt.bfloat16
F32 = mybir.dt.float32

_CACHE: dict = {}


def _build(b_shard: int):
    nc = bacc.Bacc(
        "TRN2",
        target_bir_lowering=False,
        debug=False,
        num_devices=N_CORES,
    )
    # x^T main rows pre-tiled: xTc[p, t, j] = x^T[t*128+p, j]
    xTc = nc.dram_tensor("xTc", [KT, NKT, b_shard], BF16, kind="ExternalInput")
    # tail rows of x^T, duplicated at partition offsets 0 and 32
    xtl = nc.dram_tensor("xtl", [48, b_shard], BF16, kind="ExternalInput")
    # GEMM1 weight tiles packed side by side, m-major: [128, (m*NKT+t)*MT + j]
    wta = nc.dram_tensor("wta", [KT, NKT * NMT * MT], BF16, kind="ExternalInput")
    # GEMM1 tail weights: rows 0-15 = m0, rows 32-47 = m1
    wtl = nc.dram_tensor("wtl", [48, MT], BF16, kind="ExternalInput")
    b1a = nc.dram_tensor("b1a", [MT, NMT], F32, kind="ExternalInput")
    # w2 halves side by side: [:,0:16] = rows 0-127, [:,16:32] = rows 128-255
    w2a = nc.dram_tensor("w2a", [MT, NMT * OUT_PAD], BF16, kind="ExternalInput")
    b2c = nc.dram_tensor("b2c", [OUT_PAD, 1], F32, kind="ExternalInput")
    outT = nc.dram_tensor("outT", [OUT_PAD, b_shard], F32, kind="ExternalOutput")

    relu = mybir.ActivationFunctionType.Relu
    ident = mybir.ActivationFunctionType.Identity
    chunks = [128, 128, 256, 512, 512, 512, 1024, 1024, 1024, 1024, 1024, 512, 512]
    assert sum(chunks) == b_shard
    n_ogrp = b_shard // OGRP

    with tile.TileContext(nc) as tc, ExitStack() as ctx:
        const = ctx.enter_context(tc.tile_pool(name="const", bufs=1))
        # GEMM2 constants in their own pool: sharing the bufs=1 const pool
        # with the GEMM1 weights trips a scheduler slot-wait deadlock.
        const2 = ctx.enter_context(tc.tile_pool(name="const2", bufs=1))
        opool = ctx.enter_context(tc.tile_pool(name="out", bufs=1))
        xpool = ctx.enter_context(tc.tile_pool(name="xin", bufs=3))
        hpool = ctx.enter_context(tc.tile_pool(name="h", bufs=3))
        tpool = ctx.enter_context(tc.tile_pool(name="tmp", bufs=2))
        hps = ctx.enter_context(
            tc.tile_pool(name="hps", bufs=2, space=bass.MemorySpace.PSUM)
        )
        ops = ctx.enter_context(
            tc.tile_pool(name="ops", bufs=2, space=bass.MemorySpace.PSUM)
        )
        wps = ctx.enter_context(
            tc.tile_pool(name="wps", bufs=1, space=bass.MemorySpace.PSUM)
        )

        # --- PE warm-up: garbage matmuls on a memset tile, issued before
        # any DMA-dependent work so the HAM clock gate opens during the
        # startup DMA wait.  Results land in a scratch PSUM bank that is
        # never read. ---
        wsrc = const.tile([1, 256], BF16, tag="wsrc")
        nc.gpsimd.memset(wsrc[:], 0)
        warm_ps = wps.tile([1, 256], F32, tag="warm")
        for _ in range(N_WARM):
            nc.tensor.matmul(
                warm_ps[:], wsrc[:, 0:1], wsrc[:], start=True, stop=True
            )

        # --- resident weights/biases on the ACT ring, GEMM1 weights
        # first so the first matmul group can start as soon as possible.
        HW = NKT * MT
        wt_m = []
        for m in range(NMT):
            wtile = const.tile([KT, HW], BF16, tag=f"wta{m}", name=f"wt_m{m}")
            nc.scalar.dma_start(wtile[:], wta[:, m * HW:(m + 1) * HW])
            wt_m.append(wtile)
        wtl_sb = const.tile([48, MT], BF16, tag="wtl")
        nc.scalar.dma_start(wtl_sb[:], wtl[:, :])
        b1_all = const.tile([MT, NMT], F32, tag="b1a")
        nc.scalar.dma_start(b1_all[:], b1a[:, :])
        w2_all = const2.tile([MT, NMT * OUT_PAD], BF16, tag="w2a")
        nc.scalar.dma_start(w2_all[:], w2a[:, :])
        b2_sb = const2.tile([OUT_PAD, 1], F32, tag="b2")
        nc.scalar.dma_start(b2_sb[:], b2c[:, :])

        def w_sb(t, m):
            return wt_m[m][:, t * MT:(t + 1) * MT]

        # Output accumulates in SBUF, streamed out in OGRP slabs on the
        # ACT ring (never queues behind x prefetch loads; the gpsimd
        # SWDGE ring pays a ~3us drain at kernel end).  The last slab
        # goes out in two halves to shorten the epilogue.
        oall = [
            opool.tile([OUT_PAD, OGRP], F32, tag=f"o{g}", name=f"oall{g}")
            for g in range(n_ogrp)
        ]

        # GEMM2 runs one block behind GEMM1 (software pipeline): by the
        # time it streams h, the relu that produced h is long done, so
        # the PE never stalls on the ACT semaphore.
        pending = []  # [(hb, j0, bn)] sub-blocks awaiting GEMM2

        def flush_gemm2(keep: int):
            while len(pending) > keep:
                hb2, j0, bn = pending.pop(0)
                # GEMM2 in col-group pairs (m0 -> psum rows 0-15, m1 ->
                # rows 32-47), each split K=128 into a hi half (PE rows
                # 64-127) and lo half (rows 0-63).  The hi pair occupies
                # row groups 2-3 only, so it runs concurrently with the
                # 16-row GEMM1 tails (row groups 0-1) issued just before.
                po = ops.tile([48, SUB], F32, tag="po", name="po")
                nc.tensor.matmul(
                    po[0:OUT_PAD, :bn], w2_all[:, 0:OUT_PAD], hb2[0][:, :bn],
                    start=True, stop=True,
                )
                nc.tensor.matmul(
                    po[32:32 + OUT_PAD, :bn], w2_all[:, OUT_PAD:2 * OUT_PAD],
                    hb2[1][:, :bn], start=True, stop=True,
                )
                # ACT: bias b2 onto the m0 half (reads PSUM, writes SBUF);
                # DVE: add the m1 half into the output slab.
                tsum = tpool.tile([OUT_PAD, SUB], F32, tag="tsum")
                nc.scalar.activation(
                    tsum[:, :bn], po[0:OUT_PAD, :bn], ident, bias=b2_sb[:]
                )
                g = j0 // OGRP
                o0 = j0 - g * OGRP
                nc.vector.tensor_add(
                    oall[g][:, o0:o0 + bn], tsum[:, :bn], po[32:32 + OUT_PAD, :bn]
                )
                jend = j0 + bn
                if g == n_ogrp - 1 and jend == b_shard - OGRP // 2:
                    nc.scalar.dma_start(
                        outT[:, g * OGRP:g * OGRP + OGRP // 2],
                        oall[g][:, :OGRP // 2],
                    )
                elif jend == b_shard:
                    nc.scalar.dma_start(
                        outT[:, g * OGRP + OGRP // 2:(g + 1) * OGRP],
                        oall[g][:, OGRP // 2:],
                    )
                elif jend % OGRP == 0:
                    nc.scalar.dma_start(
                        outT[:, g * OGRP:(g + 1) * OGRP], oall[g][:]
                    )

        # --- main loop over batch chunks ---
        coff = 0
        for c, csz in enumerate(chunks):
            xall = xpool.tile([KT, NKT, csz], BF16, tag="xall", name="xall")
            nc.sync.dma_start(xall[:], xTc[:, :, coff:coff + csz])
            xtail = xpool.tile([48, csz], BF16, tag="xtl", name="xtail")
            nc.sync.dma_start(xtail[:], xtl[:, coff:coff + csz])
            for s0 in range(0, csz, SUB):
                bn = min(SUB, csz - s0)
                sl = slice(s0, s0 + bn)
                ps = [
                    hps.tile([MT, SUB], F32, tag=f"ps{m}", name=f"ps{m}")
                    for m in range(NMT)
                ]
                for m in range(NMT):
                    for t in range(NKT):
                        nc.tensor.matmul(
                            ps[m][:, :bn], w_sb(t, m), xall[:, t, sl],
                            start=(t == 0), stop=False,
                        )
                # concurrent 16-row tails: row groups 0-15 and 32-47
                nc.tensor.matmul(
                    ps[0][:, :bn], wtl_sb[0:KTAIL, :], xtail[0:KTAIL, sl],
                    start=False, stop=True,
                )
                nc.tensor.matmul(
                    ps[1][:, :bn], wtl_sb[32:32 + KTAIL, :], xtail[32:32 + KTAIL, sl],
                    start=False, stop=True,
                )
                # issue the (k-2) GEMM2 right behind the tails so its hi
                # half overlaps them on the free row groups
                flush_gemm2(keep=1)
                hb = []
                for m in range(NMT):
                    h = hpool.tile([MT, SUB], BF16, tag=f"h{m}", name=f"h{m}")
                    nc.scalar.activation(
                        h[:, :bn], ps[m][:, :bn], relu, bias=b1_all[:, m:m + 1]
                    )
                    hb.append(h)
                pending.append((hb, coff + s0, bn))
            coff += csz
        flush_gemm2(keep=0)

    nc.compile()
    return nc


def _get_nc(b_shard: int = B_SHARD):
    if b_shard not in _CACHE:
        _CACHE[b_shard] = _build(b_shard)
    return _CACHE[b_shard]


def _host_prep(x, w_conv, w1, b1, w2, b2, b_shard=B_SHARD):
    """Fold conv into w1, pack weights, and lay out per-core inputs."""
    bf16 = ml_dtypes.bfloat16
    # Conv matrix Wc[784, 676]: feat[:, oi*26+oj] = sum_{di,dj} x[:, (oi+di)*28+(oj+dj)] * w_conv[di,dj]
    w_conv = np.asarray(w_conv, np.float64)
    oi = np.arange(26)
    oj = np.arange(26)
    wc = np.zeros((784, 676), np.float64)
    for di in range(3):
        for dj in range(3):
            src = ((oi[:, None] + di) * 28 + (oj[None, :] + dj)).ravel()
            dst = (oi[:, None] * 26 + oj[None, :]).ravel()
            wc[src, dst] += w_conv[di, dj]
    w_eff = (wc @ np.asarray(w1, np.float64)).astype(bf16)  # [784, 256]

    # wta[p, (m*NKT+t)*MT + j] = w_eff[t*KT+p, m*MT+j]  (m-major)
    wta = np.ascontiguousarray(
        w_eff[:NKT * KT].reshape(NKT, KT, NMT, MT).transpose(1, 2, 0, 3).reshape(KT, -1)
    )
    # tail weights at partition offsets 0 (m0) and 32 (m1)
    wtl = np.zeros((48, MT), bf16)
    wtl[0:KTAIL] = w_eff[NKT * KT:, 0:MT]
    wtl[32:32 + KTAIL] = w_eff[NKT * KT:, MT:2 * MT]
    # b1a[p, m] = b1[m*MT+p]
    b1a = np.ascontiguousarray(
        np.asarray(b1, np.float32).reshape(NMT, MT).T
    )
    # w2a[p, m*OUT_PAD + j] = w2_padded[m*MT+p, j]
    w2p = np.zeros((CH, OUT_PAD), bf16)
    w2p[:, :OUT_CH] = np.asarray(w2).astype(bf16)
    w2a = np.ascontiguousarray(
        w2p.reshape(NMT, MT, OUT_PAD).transpose(1, 0, 2).reshape(MT, -1)
    )
    b2c = np.zeros((OUT_PAD, 1), np.float32)
    b2c[:OUT_CH, 0] = np.asarray(b2, np.float32)

    x_bf = np.asarray(x).astype(bf16)  # [B, 784]
    in_maps = []
    for c in range(N_CORES):
        shard = x_bf[c * b_shard:(c + 1) * b_shard]
        shardT = np.ascontiguousarray(shard.T)  # [784, b_shard]
        # xTc[p, t, j] = shardT[t*128+p, j]
        xTc = np.ascontiguousarray(
            shardT[:NKT * KT].reshape(NKT, KT, b_shard).transpose(1, 0, 2)
        )
        xtl = np.zeros((48, b_shard), bf16)
        xtl[0:KTAIL] = shardT[NKT * KT:]
        xtl[32:32 + KTAIL] = shardT[NKT * KT:]
        in_maps.append(
            {
                "xTc": xTc,
                "xtl": xtl,
                "wta": wta,
                "wtl": wtl,
                "b1a": b1a,
                "w2a": w2a,
                "b2c": b2c,
            }
        )
    return in_maps


LAST_RESULT = None  # BassKernelResults of the most recent run (for test harness)


def kernel(x, w_conv, w1, b1, w2, b2):
    global LAST_RESULT
    nc = _get_nc()
    in_maps = _host_prep(x, w_conv, w1, b1, w2, b2)
    trace = bool(int(os.environ.get("KERNEL_TRACE", "0")))
    res = run_bass_kernel_spmd(
        nc, in_maps, list(range(N_CORES)), trace=trace,
        tmpdir=os.environ.get("KERNEL_TMPDIR") or None,
    )
    LAST_RESULT = res
    out = np.empty((B, OUT_CH), np.float32)
    for c in range(N_CORES):
        out[c * B_SHARD:(c + 1) * B_SHARD] = res.results[c]["outT"][:OUT_CH].T
    return out


# revision 19
# speedup vs baseline: 1.0847x; 1.0847x over previous
"""Trainium2 kernel for DigitConvolutionalModel.

Model: x(B,784) -> reshape(28,28) -> conv3x3 'VALID' (cross-correlation)
       -> flatten(676) -> Linear(676,256)+ReLU -> Linear(256,10).

The conv is linear, so it folds into the first Linear:
    feat = x @ Wc          (Wc: 784x676 sparse conv matrix)
    h    = relu(feat @ w1 + b1) = relu(x @ (Wc @ w1) + b1)
Device work is then two GEMMs per batch tile:
    H^T = relu(W_eff^T-tiles . x^T + b1);  out^T = w2^T . H + b2

Sharding: pure data parallel over 8 cores (8192 rows each). The host
pre-transposes each shard to x^T (contraction dim on SBUF partitions) and
casts to bf16 so the PE streams it directly; weights are replicated.

Perf structure (vs the 82us v1 baseline):
  - K tiled 6x128 + 16-row tail (was 7x112).  The two 16-row tail
    matmuls (one per 128-out m-tile) sit on disjoint PE row groups
    (partitions 0-15 / 32-47) so they execute concurrently.
  - GEMM2's two [128k,16m] matmuls go to disjoint col groups (PSUM
    partitions 0-15 / 32-47) and run concurrently; halves are summed
    on DVE, biased on ACT.
  - PE warm-up matmuls bridge the startup DMA wait so the HAM clock
    gate reaches 2.4 GHz before real work arrives.
  - One 3D DMA per chunk loads all six x k-tiles (x^T is shipped as
    [128, 6, b_shard]); few DMA queues keeps the framework's
    per-queue semaphore setup/teardown short.
  - First chunks are 256/512 columns so the first matmul starts as
    early as possible; GEMM2 pipelines one 512-block behind GEMM1.
"""

import os
from contextlib import ExitStack

import numpy as np
import ml_dtypes

import concourse.bass as bass
import concourse.tile as tile
from concourse import bacc, mybir
from concourse.bass_utils import run_bass_kernel_spmd

N_CORES = 8
B = 65536
B_SHARD = B // N_CORES  # 8192
K = 784                 # contraction dim (pixels)
KT = 128                # main k-tile partition size
NKT = 6                 # 6*128 = 768 main rows
KTAIL = K - NKT * KT    # 16 tail rows
CH = 256                # hidden channels
MT = 128                # m-tile (output channels per matmul)
NMT = CH // MT
OUT_CH = 10
OUT_PAD = 16            # padded output channels
SUB = 512               # max matmul moving free dim / PSUM bank
OGRP = 2048             # output store granularity (batch columns)
N_WARM = 20             # PE warm-up matmuls (N=256 each)
BF16 = mybir.dt.bfloat16
F32 = mybir.dt.float32

_CACHE: dict = {}


def _build(b_shard: int):
    nc = bacc.Bacc(
        "TRN2",
        target_bir_lowering=False,
        debug=False,
        num_devices=N_CORES,
    )
    # x^T main rows pre-tiled: xTc[p, t, j] = x^T[t*128+p, j]
    xTc = nc.dram_tensor("xTc", [KT, NKT, b_shard], BF16, kind="ExternalInput")
    # tail rows of x^T, duplicated at partition offsets 0 and 32
    xtl = nc.dram_tensor("xtl", [48, b_shard], BF16, kind="ExternalInput")
    # GEMM1 weight tiles packed side by side, m-major: [128, (m*NKT+t)*MT + j]
    wta = nc.dram_tensor("wta", [KT, NKT * NMT * MT], BF16, kind="ExternalInput")
    # GEMM1 tail weights: rows 0-15 = m0, rows 32-47 = m1
    wtl = nc.dram_tensor("wtl", [48, MT], BF16, kind="ExternalInput")
    b1a = nc.dram_tensor("b1a", [MT, NMT], F32, kind="ExternalInput")
    # w2 halves side by side: [:,0:16] = rows 0-127, [:,16:32] = rows 128-255
    w2a = nc.dram_tensor("w2a", [MT, NMT * OUT_PAD], BF16, kind="ExternalInput")
    b2c = nc.dram_tensor("b2c", [OUT_PAD, 1], F32, kind="ExternalInput")
    outT = nc.dram_tensor("outT", [OUT_PAD, b_shard], F32, kind="ExternalOutput")

    relu = mybir.ActivationFunctionType.Relu
    ident = mybir.ActivationFunctionType.Identity
    chunks = [256, 256, 512, 512, 512, 1024, 1024, 1024, 1024, 1024, 1024]
    assert sum(chunks) == b_shard
    n_ogrp = b_shard // OGRP

    with tile.TileContext(nc) as tc, ExitStack() as ctx:
        const = ctx.enter_context(tc.tile_pool(name="const", bufs=1))
        # GEMM2 constants in their own pool: sharing the bufs=1 const pool
        # with the GEMM1 weights trips a scheduler slot-wait deadlock.
        const2 = ctx.enter_context(tc.tile_pool(name="const2", bufs=1))
        opool = ctx.enter_context(tc.tile_pool(name="out", bufs=1))
        xpool = ctx.enter_context(tc.tile_pool(name="xin", bufs=3))
        hpool = ctx.enter_context(tc.tile_pool(name="h", bufs=3))
        tpool = ctx.enter_context(tc.tile_pool(name="tmp", bufs=2))
        hps = ctx.enter_context(
            tc.tile_pool(name="hps", bufs=2, space=bass.MemorySpace.PSUM)
        )
        ops = ctx.enter_context(
            tc.tile_pool(name="ops", bufs=2, space=bass.MemorySpace.PSUM)
        )
        wps = ctx.enter_context(
            tc.tile_pool(name="wps", bufs=1, space=bass.MemorySpace.PSUM)
        )

        # --- PE warm-up: garbage matmuls on a memset tile, issued before
        # any DMA-dependent work so the HAM clock gate opens during the
        # startup DMA wait.  Results land in a scratch PSUM bank that is
        # never read. ---
        wsrc = const.tile([1, 256], BF16, tag="wsrc")
        nc.gpsimd.memset(wsrc[:], 0)
        warm_ps = wps.tile([1, 256], F32, tag="warm")
        for _ in range(N_WARM):
            nc.tensor.matmul(
                warm_ps[:], wsrc[:, 0:1], wsrc[:], start=True, stop=True
            )

        # --- resident weights/biases on the ACT ring, GEMM1 weights
        # first so the first matmul group can start as soon as possible.
        HW = NKT * MT
        wt_m = []
        for m in range(NMT):
            wtile = const.tile([KT, HW], BF16, tag=f"wta{m}", name=f"wt_m{m}")
            nc.scalar.dma_start(wtile[:], wta[:, m * HW:(m + 1) * HW])
            wt_m.append(wtile)
        wtl_sb = const.tile([48, MT], BF16, tag="wtl")
        nc.scalar.dma_start(wtl_sb[:], wtl[:, :])
        b1_all = const.tile([MT, NMT], F32, tag="b1a")
        nc.scalar.dma_start(b1_all[:], b1a[:, :])
        w2_all = const2.tile([MT, NMT * OUT_PAD], BF16, tag="w2a")
        nc.scalar.dma_start(w2_all[:], w2a[:, :])
        b2_sb = const2.tile([OUT_PAD, 1], F32, tag="b2")
        nc.scalar.dma_start(b2_sb[:], b2c[:, :])

        def w_sb(t, m):
            return wt_m[m][:, t * MT:(t + 1) * MT]

        # Output accumulates in SBUF, streamed out in OGRP slabs on the
        # ACT ring (never queues behind x prefetch loads; the gpsimd
        # SWDGE ring pays a ~3us drain at kernel end).  The last slab
        # goes out in two halves to shorten the epilogue.
        oall = [
            opool.tile([OUT_PAD, OGRP], F32, tag=f"o{g}", name=f"oall{g}")
            for g in range(n_ogrp)
        ]

        # GEMM2 runs one block behind GEMM1 (software pipeline): by the
        # time it streams h, the relu that produced h is long done, so
        # the PE never stalls on the ACT semaphore.
        pending = []  # [(hb, j0, bn)] sub-blocks awaiting GEMM2

        def flush_gemm2(keep: int):
            while len(pending) > keep:
                hb2, j0, bn = pending.pop(0)
                # GEMM2 in col-group pairs (m0 -> psum rows 0-15, m1 ->
                # rows 32-47), each split K=128 into a hi half (PE rows
                # 64-127) and lo half (rows 0-63).  The hi pair occupies
                # row groups 2-3 only, so it runs concurrently with the
                # 16-row GEMM1 tails (row groups 0-1) issued just before.
                po = ops.tile([48, SUB], F32, tag="po", name="po")
                nc.tensor.matmul(
                    po[0:OUT_PAD, :bn], w2_all[:, 0:OUT_PAD], hb2[0][:, :bn],
                    start=True, stop=True,
                )
                nc.tensor.matmul(
                    po[32:32 + OUT_PAD, :bn], w2_all[:, OUT_PAD:2 * OUT_PAD],
                    hb2[1][:, :bn], start=True, stop=True,
                )
                # ACT: bias b2 onto the m0 half (reads PSUM, writes SBUF);
                # DVE: add the m1 half into the output slab.
                tsum = tpool.tile([OUT_PAD, SUB], F32, tag="tsum")
                nc.scalar.activation(
                    tsum[:, :bn], po[0:OUT_PAD, :bn], ident, bias=b2_sb[:]
                )
                g = j0 // OGRP
                o0 = j0 - g * OGRP
                nc.vector.tensor_add(
                    oall[g][:, o0:o0 + bn], tsum[:, :bn], po[32:32 + OUT_PAD, :bn]
                )
                jend = j0 + bn
                if g == n_ogrp - 1 and jend == b_shard - OGRP // 2:
                    nc.scalar.dma_start(
                        outT[:, g * OGRP:g * OGRP + OGRP // 2],
                        oall[g][:, :OGRP // 2],
                    )
                elif jend == b_shard:
                    nc.scalar.dma_start(
                        outT[:, g * OGRP + OGRP // 2:(g + 1) * OGRP],
                        oall[g][:, OGRP // 2:],
                    )
                elif jend % OGRP == 0:
                    nc.scalar.dma_start(
                        outT[:, g * OGRP:(g + 1) * OGRP], oall[g][:]
                    )

        # --- main loop over batch chunks ---
        coff = 0
        for c, csz in enumerate(chunks):
            xall = xpool.tile([KT, NKT, csz], BF16, tag="xall", name="xall")
            xtail = xpool.tile([48, csz], BF16, tag="xtl", name="xtail")
            # load in <=512-col slices so the first block of a chunk only
            # waits for the slice it reads (subtile deps), not the whole
            # chunk transfer
            for c0 in range(0, csz, SUB):
                cn = min(SUB, csz - c0)
                nc.sync.dma_start(
                    xall[:, :, c0:c0 + cn], xTc[:, :, coff + c0:coff + c0 + cn]
                )
                nc.sync.dma_start(
                    xtail[:, c0:c0 + cn], xtl[:, coff + c0:coff + c0 + cn]
                )
            for s0 in range(0, csz, SUB):
                bn = min(SUB, csz - s0)
                sl = slice(s0, s0 + bn)
                ps = [
                    hps.tile([MT, SUB], F32, tag=f"ps{m}", name=f"ps{m}")
                    for m in range(NMT)
                ]
                for m in range(NMT):
                    for t in range(NKT):
                        nc.tensor.matmul(
                            ps[m][:, :bn], w_sb(t, m), xall[:, t, sl],
                            start=(t == 0), stop=False,
                        )
                # concurrent 16-row tails: row groups 0-15 and 32-47
                nc.tensor.matmul(
                    ps[0][:, :bn], wtl_sb[0:KTAIL, :], xtail[0:KTAIL, sl],
                    start=False, stop=True,
                )
                nc.tensor.matmul(
                    ps[1][:, :bn], wtl_sb[32:32 + KTAIL, :], xtail[32:32 + KTAIL, sl],
                    start=False, stop=True,
                )
                # issue the (k-2) GEMM2 right behind the tails so its hi
                # half overlaps them on the free row groups
                flush_gemm2(keep=1)
                hb = []
                for m in range(NMT):
                    h = hpool.tile([MT, SUB], BF16, tag=f"h{m}", name=f"h{m}")
                    nc.scalar.activation(
                        h[:, :bn], ps[m][:, :bn], relu, bias=b1_all[:, m:m + 1]
                    )
                    hb.append(h)
                pending.append((hb, coff + s0, bn))
            coff += csz
        flush_gemm2(keep=0)

    nc.compile()
    return nc


def _get_nc(b_shard: int = B_SHARD):
    if b_shard not in _CACHE:
        _CACHE[b_shard] = _build(b_shard)
    return _CACHE[b_shard]


def _host_prep(x, w_conv, w1, b1, w2, b2, b_shard=B_SHARD):
    """Fold conv into w1, pack weights, and lay out per-core inputs."""
    bf16 = ml_dtypes.bfloat16
    # Conv matrix Wc[784, 676]: feat[:, oi*26+oj] = sum_{di,dj} x[:, (oi+di)*28+(oj+dj)] * w_conv[di,dj]
    w_conv = np.asarray(w_conv, np.float64)
    oi = np.arange(26)
    oj = np.arange(26)
    wc = np.zeros((784, 676), np.float64)
    for di in range(3):
        for dj in range(3):
            src = ((oi[:, None] + di) * 28 + (oj[None, :] + dj)).ravel()
            dst = (oi[:, None] * 26 + oj[None, :]).ravel()
            wc[src, dst] += w_conv[di, dj]
    w_eff = (wc @ np.asarray(w1, np.float64)).astype(bf16)  # [784, 256]

    # wta[p, (m*NKT+t)*MT + j] = w_eff[t*KT+p, m*MT+j]  (m-major)
    wta = np.ascontiguousarray(
        w_eff[:NKT * KT].reshape(NKT, KT, NMT, MT).transpose(1, 2, 0, 3).reshape(KT, -1)
    )
    # tail weights at partition offsets 0 (m0) and 32 (m1)
    wtl = np.zeros((48, MT), bf16)
    wtl[0:KTAIL] = w_eff[NKT * KT:, 0:MT]
    wtl[32:32 + KTAIL] = w_eff[NKT * KT:, MT:2 * MT]
    # b1a[p, m] = b1[m*MT+p]
    b1a = np.ascontiguousarray(
        np.asarray(b1, np.float32).reshape(NMT, MT).T
    )
    # w2a[p, m*OUT_PAD + j] = w2_padded[m*MT+p, j]
    w2p = np.zeros((CH, OUT_PAD), bf16)
    w2p[:, :OUT_CH] = np.asarray(w2).astype(bf16)
    w2a = np.ascontiguousarray(
        w2p.reshape(NMT, MT, OUT_PAD).transpose(1, 0, 2).reshape(MT, -1)
    )
    b2c = np.zeros((OUT_PAD, 1), np.float32)
    b2c[:OUT_CH, 0] = np.asarray(b2, np.float32)

    x_bf = np.asarray(x).astype(bf16)  # [B, 784]
    in_maps = []
    for c in range(N_CORES):
        shard = x_bf[c * b_shard:(c + 1) * b_shard]
        shardT = np.ascontiguousarray(shard.T)  # [784, b_shard]
        # xTc[p, t, j] = shardT[t*128+p, j]
        xTc = np.ascontiguousarray(
            shardT[:NKT * KT].reshape(NKT, KT, b_shard).transpose(1, 0, 2)
        )
        xtl = np.zeros((48, b_shard), bf16)
        xtl[0:KTAIL] = shardT[NKT * KT:]
        xtl[32:32 + KTAIL] = shardT[NKT * KT:]
        in_maps.append(
            {
                "xTc": xTc,
                "xtl": xtl,
                "wta": wta,
                "wtl": wtl,
                "b1a": b1a,
                "w2a": w2a,
                "b2c": b2c,
            }
        )
    return in_maps


LAST_RESULT = None  # BassKernelResults of the most recent run (for test harness)


def kernel(x, w_conv, w1, b1, w2, b2):
    global LAST_RESULT
    nc = _get_nc()
    in_maps = _host_prep(x, w_conv, w1, b1, w2, b2)
    trace = bool(int(os.environ.get("KERNEL_TRACE", "0")))
    res = run_bass_kernel_spmd(
        nc, in_maps, list(range(N_CORES)), trace=trace,
        tmpdir=os.environ.get("KERNEL_TMPDIR") or None,
    )
    LAST_RESULT = res
    out = np.empty((B, OUT_CH), np.float32)
    for c in range(N_CORES):
        out[c * B_SHARD:(c + 1) * B_SHARD] = res.results[c]["outT"][:OUT_CH].T
    return out
